# revision 1
# baseline (speedup 1.0000x reference)
"""Bass/Trainium2 kernel for nn_EquivariantPosUpdate — 8-core edge-parallel.

Structure (per core, 1024 edges in 8 tiles of 128):
  setup: load/fold weights, build replicated constant rows, identity, iota
  phase A: node projections -> DRAM proj_src/proj_dst; time-mod table -> DRAM
  phase B: per edge tile: RBF -> two radial MLPs -> per-edge TP-weight chunks
           (PE matmul) consumed by mul+reduce (DVE) -> irrep epilogues ->
           node-fusion linear -> edge-fusion TP (ss+v0 only) -> adaLN ->
           scalar head -> force -> one-hot scatter matmuls into PSUM
  final: evac accumulator -> out [2048, 3] (host sums the 8 partials)
"""
import sys
sys.path.insert(0, '/opt/trn_rl_repo')
import numpy as np
from contextlib import ExitStack

import concourse.bass as bass
import concourse.bacc as bacc
import concourse.mybir as mybir
import concourse.tile as tile
from concourse.bass import AP, IndirectOffsetOnAxis
from concourse.masks import make_identity

F32 = mybir.dt.float32
I32 = mybir.dt.int32
AX = mybir.AxisListType
OP = mybir.AluOpType
ACTF = mybir.ActivationFunctionType

N, E, G, NB = 2048, 8192, 64, 128
NC_CORES = 8
EC = E // NC_CORES          # 1024
P = 128
TILES = EC // P             # 8
M0, M1 = 64, 32
S_TP = 96
CUTOFF = 5.0
DEBUG = False
NCHUNK = N // P             # 16

# rows-packed constant layout (all replicated to 128 partitions on device)
ROWS = {}
_off = 0
for _n, _w in [('nf_g1', 64), ('nf_b1', 64), ('nf_g2', 64), ('nf_b2', 64),
               ('ef_g1', 64), ('ef_b1', 64), ('ef_g2', 64), ('ef_b2', 64),
               ('src_bs', 64), ('dst_bs', 64), ('nt_bs', 64), ('et_bs', 64),
               ('nf_bias', 96), ('ef_bias', 96), ('sp_b1', 32), ('spW2r', 32),
               ('sp_b2', 1), ('eps', 1), ('normbt', 192)]:
    ROWS[_n] = (_off, _w)
    _off += _w
RWID = _off


def rows_slice(rep, name):
    off, w = ROWS[name]
    return rep[:, off:off + w]


def ap3(t, dims, offset=0):
    """Free-dim AP with explicit [step, count] dims on an SBUF/PSUM tile."""
    base = t[:, :] if not isinstance(t, AP) else t
    ap = AP(base.tensor, base.offset + offset, [base.ap[0]] + [list(d) for d in dims])
    return ap


def build_nc():
    nc = bacc.Bacc("TRN2", target_bir_lowering=False, debug=False,
                   num_devices=NC_CORES)
    T = {}

    def din(name, shape, dtype=F32):
        T[name] = nc.dram_tensor(name, shape, dtype, kind="ExternalInput")
        return T[name]

    # --- inputs ---
    din('hn_T', [320, N]); din('he_T', [160, EC])
    din('dist', [EC, 1]); din('rvec', [EC, 3]); din('srcf', [EC, 1])
    din('srci', [EC, 1], I32); din('dsti', [EC, 1], I32); din('gidi', [EC, 1], I32)
    din('t_T', [128, G]); din('normWt', [128, 2 * S_TP])
    din('rows', [1, RWID])
    din('rbf_mean_r', [1, NB]); din('rbf_std_r', [1, NB]); din('rbf_std_c', [NB, 1])
    din('rbf_w', [1, 1]); din('rbf_b', [1, 1])
    for p in ('nf', 'ef'):
        din(p + '_W1', [NB, 64]); din(p + '_W2', [64, 64])
    din('W3nf', [64, 10240]); din('W3ef', [64, 5120])
    din('src_Ws', [128, 64]); din('dst_Ws', [128, 64])
    din('src_Wv', [64, 32]); din('dst_Wv', [64, 32])
    din('nt_Ws', [S_TP, 64]); din('nt_Wv', [128, 32])
    din('et_Ws', [64, 64]); din('et_Wv', [32, 32])
    din('sp_W1', [S_TP, 32])
    out = nc.dram_tensor('out', [N, 3], F32, kind="ExternalOutput")
    T['out'] = out
    # DRAM scratch
    T['proj_src'] = nc.dram_tensor('proj_src', [N, 160], F32)
    T['proj_dst'] = nc.dram_tensor('proj_dst', [N, 160], F32)
    T['mod_d'] = nc.dram_tensor('mod_d', [G, 2 * S_TP], F32)
    if DEBUG:
        for nm, sh in [('dbg_force', [EC, 3]), ('dbg_fs', [EC, S_TP]),
                       ('dbg_as', [EC, S_TP]), ('dbg_gsrc', [EC, 160]),
                       ('dbg_h2', [64, EC]), ('dbg_fv', [EC, 384]),
                       ('dbg_ns', [EC, 64]), ('dbg_nv', [EC, 96]),
                       ('dbg_sn', [EC, S_TP]), ('dbg_eset', [EC, 64]),
                       ('dbg_evet', [EC, 96])]:
            T[nm] = nc.dram_tensor(nm, sh, F32, kind="ExternalOutput")

    with tile.TileContext(nc) as tc:
        with ExitStack() as ctx:
            _build(ctx, tc, nc, T)
    nc.compile()
    return nc


def _build(ctx, tc, nc, T):
    consts = ctx.enter_context(tc.tile_pool(name="consts", bufs=1))
    setup = ctx.enter_context(tc.tile_pool(name="setup", bufs=2))
    sb = ctx.enter_context(tc.tile_pool(name="sb", bufs=3))
    sbq = ctx.enter_context(tc.tile_pool(name="sbq", bufs=3))
    sbg = ctx.enter_context(tc.tile_pool(name="sbg", bufs=2))
    ps = ctx.enter_context(tc.tile_pool(name="ps", bufs=4, space="PSUM"))
    psw = ctx.enter_context(tc.tile_pool(name="psw", bufs=3, space="PSUM"))
    psa = ctx.enter_context(tc.tile_pool(name="psa", bufs=1, space="PSUM"))
    dma = nc.sync.dma_start

    def load(name, shape=None, pool=consts, dt=F32):
        t = pool.tile(shape or T[name].shape, dt, tag="ld_" + name,
                      name="ld_" + name)
        dma(t[:], T[name][:])
        return t

    # ---------------- setup ----------------
    ident = consts.tile([P, P], F32)
    make_identity(nc, ident[:])
    iota_i = consts.tile([P, P], I32)
    nc.gpsimd.iota(iota_i[:], pattern=[[1, P]], base=0, channel_multiplier=0)
    iota_f = consts.tile([P, P], F32)
    nc.vector.tensor_copy(iota_f[:], iota_i[:])

    rows1 = consts.tile([1, RWID], F32)
    dma(rows1[:], T['rows'][:])
    # normbt scale-slot gets +1 (adaLN 1+scale fold)
    o_nbt = ROWS['normbt'][0]
    nc.vector.tensor_scalar_add(rows1[:, o_nbt + S_TP:o_nbt + 2 * S_TP],
                                rows1[:, o_nbt + S_TP:o_nbt + 2 * S_TP], 1.0)
    rep = consts.tile([P, RWID], F32)
    nc.gpsimd.partition_broadcast(rep[:], rows1[:])

    # RBF constants
    stdr = load('rbf_std_r', pool=setup); meanr = load('rbf_mean_r', pool=setup)
    rw = load('rbf_w', pool=setup); rb = load('rbf_b', pool=setup)
    invstd = setup.tile([1, NB], F32)
    nc.vector.reciprocal(invstd[:], stdr[:])
    arow = setup.tile([1, NB], F32)
    nc.vector.tensor_scalar(arow[:], invstd[:], rw[:, :1], 1.0 / CUTOFF,
                            op0=OP.mult, op1=OP.mult)
    minv = setup.tile([1, NB], F32)
    nc.vector.tensor_mul(minv[:], meanr[:], invstd[:])
    brow = setup.tile([1, NB], F32)
    nc.vector.scalar_tensor_tensor(brow[:], invstd[:], rb[:, :1], minv[:],
                                   op0=OP.mult, op1=OP.subtract)
    A_rep = consts.tile([P, NB], F32); B_rep = consts.tile([P, NB], F32)
    nc.gpsimd.partition_broadcast(A_rep[:], arow[:])
    nc.gpsimd.partition_broadcast(B_rep[:], brow[:])

    stdc = load('rbf_std_c', pool=setup)
    ccol = setup.tile([NB, 1], F32)
    nc.vector.reciprocal(ccol[:], stdc[:])
    nc.vector.tensor_scalar_mul(ccol[:], ccol[:], 1.0 / np.sqrt(2 * np.pi))

    W1p = consts.tile([NB, 128], F32)
    dma(W1p[:, 0:64], T['nf_W1'][:]); dma(W1p[:, 64:128], T['ef_W1'][:])
    nc.vector.tensor_scalar(W1p[:], W1p[:], ccol[:, :1], None, op0=OP.mult)
    W2nf = load('nf_W2'); W2ef = load('ef_W2')
    W3nf = load('W3nf'); W3ef = load('W3ef')

    Wsd = consts.tile([128, 128], F32)
    dma(Wsd[:, 0:64], T['src_Ws'][:]); dma(Wsd[:, 64:128], T['dst_Ws'][:])
    nc.vector.tensor_scalar_mul(Wsd[:], Wsd[:], 128.0 ** -0.5)
    Wvsd = consts.tile([64, 64], F32)
    dma(Wvsd[:, 0:32], T['src_Wv'][:]); dma(Wvsd[:, 32:64], T['dst_Wv'][:])
    nc.vector.tensor_scalar_mul(Wvsd[:], Wvsd[:], 64.0 ** -0.5)
    ntWs = load('nt_Ws'); nc.vector.tensor_scalar_mul(ntWs[:], ntWs[:], 96.0 ** -0.5)
    ntWv = load('nt_Wv'); nc.vector.tensor_scalar_mul(ntWv[:], ntWv[:], 128.0 ** -0.5)
    etWs = load('et_Ws'); nc.vector.tensor_scalar_mul(etWs[:], etWs[:], 64.0 ** -0.5)
    etWv = load('et_Wv'); nc.vector.tensor_scalar_mul(etWv[:], etWv[:], 32.0 ** -0.5)
    spW1 = load('sp_W1'); nc.vector.tensor_scalar_mul(spW1[:], spW1[:], 96.0 ** -0.5)
    normWt = load('normWt')
    tT = load('t_T')

    def evac_add(dst, src_ps, bias_ap):
        nc.vector.tensor_tensor(dst, src_ps, bias_ap, op=OP.add)

    _silu_n = [0]

    def silu(dst, src_ap, width, pool):
        _silu_n[0] += 1
        sg = pool.tile([P, width], F32, tag="silu_sg", name=f"sg_{_silu_n[0]}")
        nc.scalar.activation(sg[:], src_ap, ACTF.Sigmoid)
        nc.vector.tensor_mul(dst, sg[:], src_ap)

    # ---------------- phase A: node projections ----------------
    for c in range(NCHUNK):
        hsT = setup.tile([128, P], F32, tag="hsT")
        dma(hsT[:], T['hn_T'][0:128, c * P:(c + 1) * P])
        pp = ps.tile([P, 128], F32, tag="ps_small")
        nc.tensor.matmul(pp[:], hsT[:], Wsd[:], start=True, stop=True)
        ssb = setup.tile([P, 128], F32, tag="projs")
        evac_add(ssb[:], pp[:], rep[:, ROWS['src_bs'][0]:ROWS['src_bs'][0] + 128])
        dma(T['proj_src'][c * P:(c + 1) * P, 0:64], ssb[:, 0:64])
        dma(T['proj_dst'][c * P:(c + 1) * P, 0:64], ssb[:, 64:128])
        for x in range(3):
            hvT = setup.tile([64, P], F32, tag="hvT")
            dma(hvT[:], T['hn_T'][128 + x:320:3, c * P:(c + 1) * P])
            pv = ps.tile([P, 64], F32, tag="ps_small")
            nc.tensor.matmul(pv[:], hvT[:], Wvsd[:], start=True, stop=True)
            vsb = setup.tile([P, 64], F32, tag="projv")
            nc.scalar.copy(vsb[:], pv[:])
            dma(T['proj_src'][c * P:(c + 1) * P, 64 + 32 * x:96 + 32 * x], vsb[:, 0:32])
            dma(T['proj_dst'][c * P:(c + 1) * P, 64 + 32 * x:96 + 32 * x], vsb[:, 32:64])

    # mod table
    pm = ps.tile([G, 2 * S_TP], F32, tag="ps_small")
    nc.tensor.matmul(pm[:], tT[:], normWt[:], start=True, stop=True)
    msb = setup.tile([G, 2 * S_TP], F32)
    evac_add(msb[:], pm[:], rep[0:G, ROWS['normbt'][0]:ROWS['normbt'][0] + 2 * S_TP])
    dma(T['mod_d'][:], msb[:])

    # ---------------- phase B: edge tiles ----------------
    acc_sb = consts.tile([P, NCHUNK * 3], F32)
    nc.vector.memset(acc_sb[:], 0.0)

    for ti in range(TILES):
        e0 = ti * P
        d_col = sb.tile([P, 1], F32, tag="dcol")
        dma(d_col[:], T['dist'][e0:e0 + P, :])
        rv = sb.tile([P, 3], F32, tag="rv")
        dma(rv[:], T['rvec'][e0:e0 + P, :])
        srcf = sb.tile([P, 1], F32, tag="srcf")
        dma(srcf[:], T['srcf'][e0:e0 + P, :])
        si = sb.tile([P, 1], I32, tag="si")
        dma(si[:], T['srci'][e0:e0 + P, :])
        di = sb.tile([P, 1], I32, tag="di")
        dma(di[:], T['dsti'][e0:e0 + P, :])
        gi = sb.tile([P, 1], I32, tag="gi")
        dma(gi[:], T['gidi'][e0:e0 + P, :])

        g_src = sbg.tile([P, 160], F32, tag="gsrc")
        nc.gpsimd.indirect_dma_start(
            out=g_src[:], out_offset=None, in_=T['proj_src'][:],
            in_offset=IndirectOffsetOnAxis(ap=si[:, :1], axis=0))
        g_dst = sbg.tile([P, 160], F32, tag="gdst")
        nc.gpsimd.indirect_dma_start(
            out=g_dst[:], out_offset=None, in_=T['proj_dst'][:],
            in_offset=IndirectOffsetOnAxis(ap=di[:, :1], axis=0))
        g_mod = sbg.tile([P, 2 * S_TP], F32, tag="gmod")
        nc.gpsimd.indirect_dma_start(
            out=g_mod[:], out_offset=None, in_=T['mod_d'][:],
            in_offset=IndirectOffsetOnAxis(ap=gi[:, :1], axis=0))

        heT = sb.tile([64, P], F32, tag="heT")
        dma(heT[:], T['he_T'][0:64, e0:e0 + P])
        hevT = [sb.tile([32, P], F32, tag=f"hevT{x}", name=f"hevT{x}_{ti}")
                for x in range(3)]
        for x in range(3):
            dma(hevT[x][:], T['he_T'][64 + x:160:3, e0:e0 + P])

        # --- RBF ---
        z = sb.tile([P, NB], F32, tag="z")
        nc.vector.scalar_tensor_tensor(z[:], A_rep[:], d_col[:, :1], B_rep[:],
                                       op0=OP.mult, op1=OP.add)
        zsq = sb.tile([P, NB], F32, tag="zsq")
        nc.scalar.square(zsq[:], z[:])
        es_rbf = sb.tile([P, NB], F32, tag="esrbf")
        nc.scalar.activation(es_rbf[:], zsq[:], ACTF.Exp, scale=-0.5)
        esT_p = ps.tile([NB, P], F32, tag="ps_small")
        nc.tensor.transpose(esT_p[:], es_rbf[:], ident[:])
        esT = sb.tile([NB, P], F32, tag="esT")
        nc.scalar.copy(esT[:], esT_p[:])

        # --- radial MLPs (nf | ef share x1 matmul) ---
        x1 = ps.tile([P, 128], F32, tag="ps_small")
        nc.tensor.matmul(x1[:], esT[:], W1p[:], start=True, stop=True)

        def layer_norm(src_ap, gname, bname, dest, width):
            mu = sb.tile([P, 1], F32, tag="lnmu")
            nc.vector.tensor_reduce(mu[:], src_ap, axis=AX.X, op=OP.add)
            nc.vector.tensor_scalar_mul(mu[:], mu[:], 1.0 / width)
            cen = sb.tile([P, width], F32, tag="lncen")
            nc.vector.tensor_scalar(cen[:], src_ap, mu[:, :1], None, op0=OP.subtract)
            sqv = sb.tile([P, width], F32, tag="lnsq")
            var = sb.tile([P, 1], F32, tag="lnvar")
            nc.scalar.activation(sqv[:], cen[:], ACTF.Square, accum_out=var[:])
            std = sb.tile([P, 1], F32, tag="lnstd")
            nc.scalar.activation(std[:], var[:], ACTF.Sqrt, scale=1.0 / width,
                                 bias=rep[:, ROWS['eps'][0]:ROWS['eps'][0] + 1])
            rstd = sb.tile([P, 1], F32, tag="lnrstd")
            nc.vector.reciprocal(rstd[:], std[:])
            nc.vector.scalar_tensor_tensor(dest, cen[:], rstd[:, :1],
                                           rows_slice(rep, gname),
                                           op0=OP.mult, op1=OP.mult)
            nc.vector.tensor_tensor(dest, dest, rows_slice(rep, bname), op=OP.add)

        h2T = {}
        for ri, p in enumerate(('nf', 'ef')):
            hln = sb.tile([P, 64], F32, tag=f"hln{p}")
            layer_norm(x1[:, 64 * ri:64 * ri + 64], p + '_g1', p + '_b1', hln[:], 64)
            h1 = sb.tile([P, 64], F32, tag=f"h1{p}")
            silu(h1[:], hln[:], 64, sb)
            h1T_p = ps.tile([64, P], F32, tag="ps_small")
            nc.tensor.transpose(h1T_p[:], h1[:], ident[:])
            h1T = sb.tile([64, P], F32, tag=f"h1T{p}")
            nc.scalar.copy(h1T[:], h1T_p[:])
            x2 = ps.tile([P, 64], F32, tag="ps_small")
            nc.tensor.matmul(x2[:], h1T[:], (W2nf if p == 'nf' else W2ef)[:],
                             start=True, stop=True)
            h2ln = sb.tile([P, 64], F32, tag=f"h2ln{p}")
            layer_norm(x2[:, :], p + '_g2', p + '_b2', h2ln[:], 64)
            h2 = sb.tile([P, 64], F32, tag=f"h2{p}")
            silu(h2[:], h2ln[:], 64, sb)
            h2T_p = ps.tile([64, P], F32, tag="ps_small")
            nc.tensor.transpose(h2T_p[:], h2[:], ident[:])
            h2T[p] = sb.tile([64, P], F32, tag=f"h2T{p}", name=f"h2T{p}_{ti}")
            nc.scalar.copy(h2T[p][:], h2T_p[:])

        # --- edge transform (es/ev from h_edge) ---
        pe_s = ps.tile([P, 64], F32, tag="ps_small")
        nc.tensor.matmul(pe_s[:], heT[:], etWs[:], start=True, stop=True)
        es_et = sb.tile([P, 64], F32, tag="eset")
        evac_add(es_et[:], pe_s[:], rows_slice(rep, 'et_bs'))
        pe_v = ps.tile([P, 96], F32, tag="ps_small")
        for x in range(3):
            nc.tensor.matmul(pe_v[:, 32 * x:32 * x + 32], hevT[x][:], etWv[:],
                             start=True, stop=True, skip_group_check=True)
        ev_et = sb.tile([P, 96], F32, tag="evet")
        nc.scalar.copy(ev_et[:], pe_v[:])

        s1 = g_src[:, 0:64]; v1 = g_src[:, 64:160]
        s2 = g_dst[:, 0:64]; v2 = g_dst[:, 64:160]

        # --- dtp helper: consume one radial's W3 stream ---
        def dtp(h2T_sb, W3, s_in, v_in, full):
            """Returns dict of bilinear buffers."""
            fl = 'f' if full else 'h'
            r = {}
            r['bil_ss'] = sbq.tile([P, 64], F32, tag="bilss", name=f"bilss{fl}_{ti}")
            if full:
                r['bsv'] = sbq.tile([P, 192], F32, tag="bsv", name=f"bsv_{ti}")
                r['bvs'] = sbq.tile([P, 32], F32, tag="bvs", name=f"bvs_{ti}")
                r['cbuf'] = sbq.tile([P, 96], F32, tag="cbuf", name=f"cbuf_{ti}")
            r['bv0'] = sbq.tile([P, 96], F32, tag="bv0", name=f"bv0{fl}_{ti}")
            nchunks = 20 if full else 10
            for c in range(nchunks):
                pw = psw.tile([P, 512], F32)
                nc.tensor.matmul(pw[:], h2T_sb[:], W3[:, 512 * c:512 * c + 512],
                                 start=True, stop=True)
                if full:
                    kind = ('ss' if c < 8 else 'sv' if c < 12 else
                            'vs' if c < 16 else 'v0' if c < 18 else 'v1')
                    ci = {'ss': c, 'sv': c - 8, 'vs': c - 12,
                          'v0': c - 16, 'v1': c - 18}[kind]
                else:
                    kind = 'ss' if c < 8 else 'v0'
                    ci = c if c < 8 else c - 8
                if kind in ('ss', 'vs'):
                    # chunk = 8 u x 64 v ; mul by s_in bcast over u, reduce v
                    q = sbq.tile([P, 512], F32, tag="qs")
                    nc.vector.tensor_tensor(
                        ap3(q, [[64, 8], [1, 64]]),
                        ap3(pw, [[64, 8], [1, 64]]),
                        ap3(s_in, [[0, 8], [1, 64]]), op=OP.mult)
                    dst = r['bil_ss'] if kind == 'ss' else r['bvs']
                    nc.vector.tensor_reduce(
                        dst[:, 8 * ci:8 * ci + 8],
                        ap3(q, [[64, 8], [1, 64]]), axis=AX.X, op=OP.add)
                else:
                    # chunk = 16 u x 32 v ; q [e,(16u,3x,32v)], reduce v
                    q = sbq.tile([P, 1536], F32, tag="qv")
                    nc.vector.tensor_tensor(
                        ap3(q, [[96, 16], [32, 3], [1, 32]]),
                        ap3(pw, [[32, 16], [0, 3], [1, 32]]),
                        ap3(v_in, [[0, 16], [32, 3], [1, 32]]), op=OP.mult)
                    dst = r['bsv'] if kind == 'sv' else (
                        r['bv0'] if kind == 'v0' else r['cbuf'])
                    nc.vector.tensor_reduce(
                        ap3(dst, [[3, 16], [1, 3]], offset=48 * ci),
                        ap3(q, [[96, 16], [32, 3], [1, 32]]), axis=AX.X, op=OP.add)
            return r

        # ---- dtp1: (s1,v1) x (s2,v2), weights from h2nf ----
        b1r = dtp(h2T['nf'], W3nf, s2, v2, full=True)
        fs = sbq.tile([P, 96], F32, tag="fs")
        fv = sbq.tile([P, 384], F32, tag="fv")
        # out_ss = s1 * bil_ss / 8
        nc.vector.scalar_tensor_tensor(fs[:, 0:64], b1r['bil_ss'][:], 0.125,
                                       s1, op0=OP.mult, op1=OP.mult)
        # out_v0 = sum_x v1*(bv0)/sqrt(96)
        t96 = sbq.tile([P, 96], F32, tag="t96")
        nc.vector.scalar_tensor_tensor(
            ap3(t96, [[3, 32], [1, 3]]),
            ap3(v1, [[1, 32], [32, 3]]), 96.0 ** -0.5,
            ap3(b1r['bv0'], [[3, 32], [1, 3]]), op0=OP.mult, op1=OP.mult)
        nc.vector.tensor_reduce(fs[:, 64:96], ap3(t96, [[3, 32], [1, 3]]),
                                axis=AX.X, op=OP.add)
        nc.vector.tensor_tensor(fs[:], fs[:], rows_slice(rep, 'nf_bias'), op=OP.add)
        # fv sv region: s1 * bsv / sqrt(32)
        nc.vector.scalar_tensor_tensor(
            ap3(fv, [[128, 3], [1, 64]]),
            ap3(b1r['bsv'], [[1, 3], [3, 64]]), 32.0 ** -0.5,
            ap3(s1, [[0, 3], [1, 64]]), op0=OP.mult, op1=OP.mult)
        # fv vs region: v1 * bvs / 8
        nc.vector.scalar_tensor_tensor(
            ap3(fv, [[128, 3], [1, 32]], offset=64),
            ap3(v1, [[32, 3], [1, 32]]), 0.125,
            ap3(b1r['bvs'], [[0, 3], [1, 32]]), op0=OP.mult, op1=OP.mult)
        # fv v1-term region: cross(v1, c)/8
        for x in range(3):
            y, zz = (x + 1) % 3, (x + 2) % 3
            ta = sbq.tile([P, 32], F32, tag="crossa")
            nc.vector.scalar_tensor_tensor(
                ta[:], v1[:, 32 * y:32 * y + 32], 0.125,
                ap3(b1r['cbuf'], [[3, 32]], offset=zz), op0=OP.mult, op1=OP.mult)
            tb = sbq.tile([P, 32], F32, tag="crossb")
            nc.vector.scalar_tensor_tensor(
                tb[:], v1[:, 32 * zz:32 * zz + 32], 0.125,
                ap3(b1r['cbuf'], [[3, 32]], offset=y), op0=OP.mult, op1=OP.mult)
            nc.vector.tensor_sub(fv[:, 128 * x + 96:128 * x + 128], ta[:], tb[:])

        # ---- node-fusion linear ----
        fsT_p = ps.tile([96, P], F32, tag="ps_small")
        nc.tensor.transpose(fsT_p[:], fs[:], ident[:])
        fsT = sbq.tile([96, P], F32, tag="fsT")
        nc.scalar.copy(fsT[:], fsT_p[:])
        ns_p = ps.tile([P, 64], F32, tag="ps_small")
        nc.tensor.matmul(ns_p[:], fsT[:], ntWs[:], start=True, stop=True)
        ns = sbq.tile([P, 64], F32, tag="ns")
        evac_add(ns[:], ns_p[:], rows_slice(rep, 'nt_bs'))
        nv = sbq.tile([P, 96], F32, tag="nv")
        for x in range(3):
            fvT_p = ps.tile([128, P], F32, tag="ps_small")
            nc.tensor.transpose(fvT_p[:], fv[:, 128 * x:128 * x + 128], ident[:])
            fvT = sbq.tile([128, P], F32, tag="fvT")
            nc.scalar.copy(fvT[:], fvT_p[:])
            nv_p = ps.tile([P, 32], F32, tag="ps_small")
            nc.tensor.matmul(nv_p[:], fvT[:], ntWv[:], start=True, stop=True)
            nc.scalar.copy(nv[:, 32 * x:32 * x + 32], nv_p[:])

        # ---- dtp2 ----
        b2r = dtp(h2T['ef'], W3ef, es_et[:, :], ev_et[:, :], full=False)
        as_ = sbq.tile([P, 96], F32, tag="as")
        nc.vector.scalar_tensor_tensor(as_[:, 0:64], b2r['bil_ss'][:], 0.125,
                                       ns[:], op0=OP.mult, op1=OP.mult)
        t96b = sbq.tile([P, 96], F32, tag="t96b")
        nc.vector.scalar_tensor_tensor(
            ap3(t96b, [[3, 32], [1, 3]]),
            ap3(nv, [[1, 32], [32, 3]]), 96.0 ** -0.5,
            ap3(b2r['bv0'], [[3, 32], [1, 3]]), op0=OP.mult, op1=OP.mult)
        nc.vector.tensor_reduce(as_[:, 64:96], ap3(t96b, [[3, 32], [1, 3]]),
                                axis=AX.X, op=OP.add)
        nc.vector.tensor_tensor(as_[:], as_[:], rows_slice(rep, 'ef_bias'), op=OP.add)

        # ---- adaLN ----
        mu = sb.tile([P, 1], F32, tag="amu")
        nc.vector.tensor_reduce(mu[:], as_[:], axis=AX.X, op=OP.add)
        nc.vector.tensor_scalar_mul(mu[:], mu[:], 1.0 / S_TP)
        cen = sbq.tile([P, S_TP], F32, tag="acen")
        nc.vector.tensor_scalar(cen[:], as_[:], mu[:, :1], None, op0=OP.subtract)
        sqv = sbq.tile([P, S_TP], F32, tag="asq")
        var = sb.tile([P, 1], F32, tag="avar")
        nc.scalar.activation(sqv[:], cen[:], ACTF.Square, accum_out=var[:])
        std = sb.tile([P, 1], F32, tag="astd")
        nc.scalar.activation(std[:], var[:], ACTF.Sqrt, scale=1.0 / S_TP,
                             bias=rep[:, ROWS['eps'][0]:ROWS['eps'][0] + 1])
        rstd = sb.tile([P, 1], F32, tag="arstd")
        nc.vector.reciprocal(rstd[:], std[:])
        s_n = sbq.tile([P, S_TP], F32, tag="sn")
        nc.vector.scalar_tensor_tensor(s_n[:], cen[:], rstd[:, :1],
                                       g_mod[:, S_TP:2 * S_TP],
                                       op0=OP.mult, op1=OP.mult)
        nc.vector.tensor_tensor(s_n[:], s_n[:], g_mod[:, 0:S_TP], op=OP.add)

        # ---- scalar head ----
        snT_p = ps.tile([S_TP, P], F32, tag="ps_small")
        nc.tensor.transpose(snT_p[:], s_n[:], ident[:])
        snT = sbq.tile([S_TP, P], F32, tag="snT")
        nc.scalar.copy(snT[:], snT_p[:])
        hd_p = ps.tile([P, 32], F32, tag="ps_small")
        nc.tensor.matmul(hd_p[:], snT[:], spW1[:], start=True, stop=True)
        hd = sb.tile([P, 32], F32, tag="hd")
        evac_add(hd[:], hd_p[:], rows_slice(rep, 'sp_b1'))
        silu(hd[:], hd[:], 32, sb)
        swt = sb.tile([P, 32], F32, tag="swt")
        nc.vector.tensor_tensor(swt[:], hd[:], rows_slice(rep, 'spW2r'), op=OP.mult)
        swr = sb.tile([P, 1], F32, tag="swr")
        nc.vector.tensor_reduce(swr[:], swt[:], axis=AX.X, op=OP.add)
        sw = sb.tile([P, 1], F32, tag="sw")
        nc.vector.tensor_scalar(sw[:], swr[:], 32.0 ** -0.5,
                                rep[:, ROWS['sp_b2'][0]:ROWS['sp_b2'][0] + 1],
                                op0=OP.mult, op1=OP.add)
        den = sb.tile([P, 1], F32, tag="den")
        nc.vector.scalar_tensor_tensor(den[:], d_col[:], 1.0, d_col[:],
                                       op0=OP.add, op1=OP.mult)
        rden = sb.tile([P, 1], F32, tag="rden")
        nc.vector.reciprocal(rden[:], den[:])
        coef = sb.tile([P, 1], F32, tag="coef")
        nc.vector.tensor_mul(coef[:], sw[:], rden[:])
        force = sb.tile([P, 3], F32, tag="force")
        nc.vector.tensor_scalar(force[:], rv[:], coef[:, :1], None, op0=OP.mult)

        if DEBUG:
            dma(T['dbg_force'][e0:e0 + P, :], force[:])
            dma(T['dbg_fs'][e0:e0 + P, :], fs[:])
            dma(T['dbg_as'][e0:e0 + P, :], as_[:])
            dma(T['dbg_gsrc'][e0:e0 + P, :], g_src[:])
            dma(T['dbg_h2'][:, e0:e0 + P], h2T['nf'][:])
            dma(T['dbg_fv'][e0:e0 + P, :], fv[:])
            dma(T['dbg_ns'][e0:e0 + P, :], ns[:])
            dma(T['dbg_nv'][e0:e0 + P, :], nv[:])
            dma(T['dbg_sn'][e0:e0 + P, :], s_n[:])
            dma(T['dbg_eset'][e0:e0 + P, :], es_et[:])
            dma(T['dbg_evet'][e0:e0 + P, :], ev_et[:])

        # ---- scatter: one-hot matmuls into persistent accumulator ----
        acc_p = psa.tile([P, NCHUNK * 3], F32)
        for ch in range(NCHUNK):
            ssh = sb.tile([P, 1], F32, tag="ssh")
            nc.vector.tensor_scalar_add(ssh[:], srcf[:], float(-P * ch))
            oh = sb.tile([P, P], F32, tag="oh")
            nc.vector.tensor_scalar(oh[:], iota_f[:], ssh[:, :1], None,
                                    op0=OP.is_equal)
            nc.tensor.matmul(acc_p[:, 3 * ch:3 * ch + 3], oh[:], force[:],
                             start=True, stop=True, skip_group_check=True)
        nc.vector.tensor_add(acc_sb[:], acc_sb[:], acc_p[:])

    # ---------------- final: evac accumulator ----------------
    for ch in range(NCHUNK):
        dma(T['out'][ch * P:(ch + 1) * P, :], acc_sb[:, 3 * ch:3 * ch + 3])


# ======================= host side =======================

def host_prep(inp):
    inp = {k: np.asarray(v) for k, v in inp.items()}
    src = inp['edge_index'][0].astype(np.int32)
    dst = inp['edge_index'][1].astype(np.int32)
    perm = np.argsort(src, kind='stable')
    src, dst = src[perm], dst[perm]
    gid = inp['batch'].astype(np.int32)[src]
    h_edge = inp['h_edge'][perm]
    dist = inp['distance'][perm].astype(np.float32)
    rvec = inp['relative_vec'][perm].astype(np.float32)

    rows = np.zeros(RWID, np.float32)

    def setr(name, val):
        off, w = ROWS[name]
        rows[off:off + w] = val
    for p in ('nf', 'ef'):
        for q in ('g1', 'b1', 'g2', 'b2'):
            setr(f'{p}_{q}', inp[f'{p}_{q}'])
    setr('src_bs', inp['src_bs']); setr('dst_bs', inp['dst_bs'])
    setr('nt_bs', inp['nt_bs']); setr('et_bs', inp['et_bs'])
    setr('nf_bias', inp['nf_bias']); setr('ef_bias', inp['ef_bias'])
    setr('sp_b1', inp['sp_b1']); setr('spW2r', inp['sp_W2'][:, 0])
    rows[ROWS['sp_b2'][0]] = inp['sp_b2'][0]
    rows[ROWS['eps'][0]] = 1e-5
    setr('normbt', inp['norm_bt'][:2 * S_TP])

    W3ef = inp['ef_W3']
    shared = dict(
        hn_T=np.ascontiguousarray(inp['h_node'].T),
        t_T=np.ascontiguousarray(inp['t'].T),
        normWt=np.ascontiguousarray(inp['norm_Wt'][:, :2 * S_TP]),
        rows=rows.reshape(1, -1),
        rbf_mean_r=inp['rbf_mean'].reshape(1, -1).astype(np.float32),
        rbf_std_r=inp['rbf_std'].reshape(1, -1).astype(np.float32),
        rbf_std_c=inp['rbf_std'].reshape(-1, 1).astype(np.float32),
        rbf_w=inp['rbf_w'].reshape(1, 1).astype(np.float32),
        rbf_b=inp['rbf_b'].reshape(1, 1).astype(np.float32),
        nf_W1=inp['nf_W1'], nf_W2=inp['nf_W2'],
        ef_W1=inp['ef_W1'], ef_W2=inp['ef_W2'],
        W3nf=np.ascontiguousarray(inp['nf_W3']),
        W3ef=np.ascontiguousarray(
            np.concatenate([W3ef[:, :4096], W3ef[:, 8192:9216]], axis=1)),
        src_Ws=inp['src_Ws'], dst_Ws=inp['dst_Ws'],
        src_Wv=inp['src_Wv'], dst_Wv=inp['dst_Wv'],
        nt_Ws=inp['nt_Ws'], nt_Wv=inp['nt_Wv'],
        et_Ws=inp['et_Ws'], et_Wv=inp['et_Wv'],
        sp_W1=inp['sp_W1'],
    )
    shared = {k: np.ascontiguousarray(v, dtype=np.float32) for k, v in shared.items()}

    in_maps = []
    for c in range(NC_CORES):
        sl = slice(c * EC, (c + 1) * EC)
        m = dict(shared)
        m['he_T'] = np.ascontiguousarray(h_edge[sl].T, dtype=np.float32)
        m['dist'] = dist[sl].reshape(-1, 1)
        m['rvec'] = rvec[sl]
        m['srcf'] = src[sl].reshape(-1, 1).astype(np.float32)
        m['srci'] = np.ascontiguousarray(src[sl].reshape(-1, 1))
        m['dsti'] = np.ascontiguousarray(dst[sl].reshape(-1, 1))
        m['gidi'] = np.ascontiguousarray(gid[sl].reshape(-1, 1))
        in_maps.append(m)
    return in_maps


_CACHED_NC = None


def kernel(**inputs):
    global _CACHED_NC
    from concourse.bass_utils import run_bass_kernel_spmd
    if _CACHED_NC is None:
        _CACHED_NC = build_nc()
    in_maps = host_prep(inputs)
    res = run_bass_kernel_spmd(_CACHED_NC, in_maps, list(range(NC_CORES)))
    out = np.zeros((N, 3), np.float32)
    for r in res.results:
        out += r['out']
    return out



# revision 41
# speedup vs baseline: 190.1038x; 190.1038x over previous
"""Bass/Trainium2 kernel for nn_EquivariantPosUpdate — 8-core edge-parallel.

v2 (per core, 1024 edges in 8 tiles of 128), transposed-dtp design:
  phase R: RBF + both radial MLPs for all 1024 edges, ACT funcs batched
           (2 activation-table loads total); h2T [128,1024] fp16 in SBUF
  phase A: node projections -> DRAM proj [N, 320] fp16, mod table fp16
  per edge tile: dup-gathers + fp16 PE transposes build replicated dst/edge
    operands; W3 chunk matmuls (fp16) -> ACT evac-cast fp16; DVE mult (2x);
    PE reduce-matmuls with selection stationaries accumulate bilinears in
    PSUM; transposed epilogues feed nt-linear / dtp2 / adaLN / head matmuls
    directly; one-hot scatter matmuls into a persistent PSUM accumulator.
  host folds: all irrep scales, dst/edge-transform (et) linear + biases into
    W3 streams; RBF affine into per-basis columns; gaussian norm into W1.
"""
import sys
sys.path.insert(0, '/opt/trn_rl_repo')
import numpy as np
from contextlib import ExitStack

import concourse.bass as bass
import concourse.bacc as bacc
import concourse.mybir as mybir
import concourse.tile as tile
from concourse.bass import AP, IndirectOffsetOnAxis
from concourse.masks import make_identity

F32 = mybir.dt.float32
F16 = mybir.dt.float16
I32 = mybir.dt.int32
AX = mybir.AxisListType
OP = mybir.AluOpType
ACTF = mybir.ActivationFunctionType

N, E, G, NB = 2048, 8192, 64, 128
NC_CORES = 8
EC = E // NC_CORES          # 1024
P = 128
TILES = EC // P             # 8
M0, M1 = 64, 32
S_TP = 96
CUTOFF = 5.0
NCHUNK = N // P             # 16

# rows-packed fp32 LN constants (g/b packed [nf64|ef64] per slot)
ROWS = {'g1': 0, 'b1': 128, 'g2': 256, 'b2': 384}
RWID = 512

# biascols [128, 8] fp32 column index
BCOL = {'nf_bias': 0, 'ef_bias': 1, 'nt_bs': 2, 'src_bs': 3, 'sp_b1': 4}

# dtp regions: (name, w3_col_off, n_chunks, sel_key, sel_width, n_x)
REG_NF = [
    ('ss', 0,    32, 'ss', 64, 1),
    ('sv', 4096, 16, 'sv', 64, 3),
    ('vs', 6144, 16, 'vs', 32, 1),
    ('v0', 8192, 8,  'v0', 32, 3),
    ('v1', 9216, 8,  'v0', 32, 3),
]
REG_EF = [
    ('ss2', 0,    32, 'ss', 64, 1),
    ('v02', 4096, 8,  'v0', 32, 3),
]
SEL_OFF = {'ss': 0, 'sv': 2048, 'vs': 3072, 'v0': 3584}
SEL_W = 3840


def ap3(t, dims, offset=0):
    """Free-dim AP with explicit [step, count] dims on an SBUF/PSUM tile."""
    base = t[:, :] if not isinstance(t, AP) else t
    ap = AP(base.tensor, base.offset + offset, [base.ap[0]] + [list(d) for d in dims])
    return ap


def build_nc(reps=1):
    nc = bacc.Bacc("TRN2", target_bir_lowering=False, debug=False,
                   num_devices=NC_CORES)
    T = {}

    def din(name, shape, dtype=F32):
        T[name] = nc.dram_tensor(name, shape, dtype, kind="ExternalInput")
        return T[name]

    din('hn_T', [320, N], F16)          # rows: hs 0-127 | hv_x0 | hv_x1 | hv_x2
    din('heDup', [512, EC], F16)        # rows: hes x2 | hev0 x4 | hev1 x4 | hev2 x4
    din('distR', [1, EC])
    din('rvecb', [EC, 3], F16)
    din('eidx', [EC, 3], I32)           # cols: src | dst | gid
    din('emeta', [EC, 2])               # cols: srcf | dist
    din('t_T', [128, G]); din('normWt', [128, 2 * S_TP]); din('normbt', [1, 2 * S_TP])
    din('rows', [1, RWID])
    din('biascols', [128, 8])
    din('acol', [NB, 1]); din('bcol', [NB, 1])
    din('W1p', [NB, 128], F16)   # cols: nf 0-63 | ef 64-127
    din('W2nf', [64, 64], F16); din('W2ef', [64, 64], F16)
    din('W3nf', [64, 10240], F16); din('W3ef', [64, 5120], F16)
    din('C1ss', [64, 64], F16); din('C1vs', [64, 32], F16); din('C2ss', [64, 64], F16)
    din('SEL', [128, SEL_W], F16)
    din('Wsd3', [128, 192], F16); din('Wv3', [64, 160], F16)
    din('ntWs', [S_TP, 64], F16); din('ntWv', [128, 32], F16)
    din('spW1', [S_TP, 32], F16); din('spW2c', [32, 1], F16)
    din('sp_b2', [1, 1])
    din('iotaR', [1, N], F16)
    T['out'] = nc.dram_tensor('out', [N, 3], F32, kind="ExternalOutput")
    T['proj_s'] = nc.dram_tensor('proj_s', [N, 160], F16)
    T['proj_d'] = nc.dram_tensor('proj_d', [N, 512], F16)
    T['mod_d'] = nc.dram_tensor('mod_d', [G, 2 * S_TP], F16)

    with tile.TileContext(nc) as tc:
        for rep in range(reps):
            with ExitStack() as ctx:
                _build(ctx, tc, nc, T, rep=rep)
    nc.compile()
    return nc


def _build(ctx, tc, nc, T, rep=0):
    Rp = f"r{rep}_"
    consts = ctx.enter_context(tc.tile_pool(name=Rp + "consts", bufs=1))
    setup = ctx.enter_context(tc.tile_pool(name=Rp + "setup", bufs=2))
    sb = ctx.enter_context(tc.tile_pool(name=Rp + "sb", bufs=1))
    sbq = ctx.enter_context(tc.tile_pool(name=Rp + "sbq", bufs=3))
    sbg = ctx.enter_context(tc.tile_pool(name=Rp + "sbg", bufs=2))
    big1 = ctx.enter_context(tc.tile_pool(name=Rp + "big1", bufs=1))
    sboh = ctx.enter_context(tc.tile_pool(name=Rp + "sboh", bufs=2))
    dma = nc.sync.dma_start

    def load(name, pool=consts, dt=None, eng=None):
        t = pool.tile(T[name].shape, dt or T[name].dtype, tag="ld_" + name,
                      name=Rp + "ld_" + name)
        (eng or nc.sync).dma_start(t[:], T[name][:])
        return t

    # ---------------- setup ----------------
    ident16 = consts.tile([P, P], F16)
    make_identity(nc, ident16[:])
    ones96 = consts.tile([S_TP, 1], F16)
    nc.vector.memset(ones96[:], 1.0)
    one32 = consts.tile([1, 1], F32)
    nc.vector.memset(one32[:], 1.0)

    rows1 = setup.tile([1, RWID], F32, tag="rows1")
    dma(rows1[:], T['rows'][:])
    rws = consts.tile([P, RWID], F32)
    nc.gpsimd.partition_broadcast(rws[:], rows1[:])
    bcols = load('biascols')
    acol = load('acol'); bcol = load('bcol')
    W1p = load('W1p'); W2nf = load('W2nf'); W2ef = load('W2ef')
    W3nf = load('W3nf', eng=nc.scalar); W3ef = load('W3ef', eng=nc.gpsimd)
    C1ss = load('C1ss'); C1vs = load('C1vs'); C2ss = load('C2ss')
    SEL = load('SEL', eng=nc.scalar)
    Wsd3 = load('Wsd3'); Wv3 = load('Wv3')
    ntWs = load('ntWs'); ntWv = load('ntWv')
    spW1 = load('spW1'); spW2c = load('spW2c')
    normWt = load('normWt'); tT = load('t_T')
    sp_b2 = load('sp_b2')
    distR = load('distR')

    nbt1 = setup.tile([1, 2 * S_TP], F32, tag="nbt1")
    dma(nbt1[:], T['normbt'][:])
    nbt = consts.tile([G, 2 * S_TP], F32)
    nc.gpsimd.partition_broadcast(nbt[:], nbt1[:])

    iota_r = setup.tile([1, N], F16, tag="iota_r")
    dma(iota_r[:], T['iotaR'][:])
    iota16 = consts.tile([P, N], F16)
    nc.gpsimd.partition_broadcast(iota16[:], iota_r[:])

    def bias_col(name, n):
        c = BCOL[name]
        return bcols[0:n, c:c + 1]


    def rsqrt_ops(eng, pool, dst, src_ap, shape, tagp, eps=1e-5):
        """dst = (src+eps)^(-1/2) via bit-trick seed + 2 Newton iterations."""
        v = pool.tile(shape, F32, tag=tagp + "v")
        eng.tensor_scalar(v[:], src_ap, eps, None, op0=OP.add)
        vi = pool.tile(shape, I32, tag=tagp + "vi")
        eng.tensor_scalar(vi[:], v[:].bitcast(I32), 1, None,
                          op0=OP.arith_shift_right)
        eng.tensor_scalar(vi[:], vi[:], -1, 0x5F3759DF, op0=OP.mult, op1=OP.add)
        h = pool.tile(shape, F32, tag=tagp + "h")
        eng.tensor_scalar(h[:], v[:], 0.5, None, op0=OP.mult)
        y = dst
        eng.tensor_copy(y, vi[:].bitcast(F32))
        t = pool.tile(shape, F32, tag=tagp + "t")
        for _ in range(2):
            eng.tensor_tensor(t[:], y, y, op=OP.mult)
            eng.tensor_tensor(t[:], t[:], h[:], op=OP.mult)
            eng.tensor_scalar(t[:], t[:], -1.0, 1.5, op0=OP.mult, op1=OP.add)
            eng.tensor_tensor(y, y, t[:], op=OP.mult)

    # SBUF scatter accumulator (summed across tiles on DVE)
    acc_sb = consts.tile([P, NCHUNK * 3], F32)
    nc.vector.memset(acc_sb[:], 0.0)

    # ---------------- phase R + A (own psum pools, closed before dtp) -----
    with tc.tile_pool(name=Rp + "psR", bufs=1, space="PSUM") as psR, \
         tc.tile_pool(name=Rp + "psT", bufs=1, space="PSUM") as psT, \
         tc.tile_pool(name=Rp + "psPA", bufs=1, space="PSUM") as psPA:

        distRep = big1.tile([P, EC], F32, tag="distRep")
        nc.gpsimd.partition_broadcast(distRep[:], distR[:])
        zT = setup.tile([NB, EC], F32, tag="zz")
        nc.vector.tensor_scalar(zT[:], distRep[:], acol[:, :1], bcol[:, :1],
                                op0=OP.mult, op1=OP.add)
        zsqT = setup.tile([NB, EC], F32, tag="zz", name=Rp + "zsqT")
        nc.scalar.activation(zsqT[:], zT[:], ACTF.Square)
        esT = big1.tile([NB, EC], F16, tag="esT")
        nc.scalar.activation(esT[:], zsqT[:], ACTF.Exp, scale=-0.5)

        xx1 = psR.tile([P, EC], F32, tag="xx", name=Rp + "xx1")
        for t in range(TILES):
            nc.tensor.matmul(xx1[:, t * 64:(t + 1) * 64],
                             esT[:, t * P:(t + 1) * P], W1p[:, 0:64],
                             start=True, stop=True, skip_group_check=True)
            nc.tensor.matmul(xx1[:, 512 + t * 64:512 + (t + 1) * 64],
                             esT[:, t * P:(t + 1) * P], W1p[:, 64:128],
                             start=True, stop=True, skip_group_check=True)

        def ln_silu_half(xx, xoff, goff, out16, oslice, tag):
            # one radial's half: xx cols [xoff, xoff+512), 8 segments of 64
            xap = ap3(xx, [[64, 8], [1, 64]], offset=xoff)
            mu = sb.tile([P, 8], F32, tag=tag + "mu")
            nc.vector.tensor_reduce(mu[:], xap, axis=AX.X, op=OP.add)
            nc.vector.tensor_scalar_mul(mu[:], mu[:], 1.0 / 64)
            cen = sb.tile([P, 512], F32, tag=tag + "cen")
            nc.vector.tensor_tensor(cen[:], xap,
                                    ap3(mu, [[1, 8], [0, 64]]), op=OP.subtract)
            sq = sb.tile([P, 512], F32, tag=tag + "sq")
            nc.vector.tensor_mul(sq[:], cen[:], cen[:])
            var = sb.tile([P, 8], F32, tag=tag + "var")
            nc.vector.tensor_reduce(var[:], ap3(sq, [[64, 8], [1, 64]]),
                                    axis=AX.X, op=OP.add)
            rstd = sb.tile([P, 8], F32, tag=tag + "rstd")
            nc.vector.tensor_scalar(rstd[:], var[:], 1.0 / 64, None, op0=OP.mult)
            rstd2 = sb.tile([P, 8], F32, tag=tag + "rstd2")
            rsqrt_ops(nc.vector, sb, rstd2[:], rstd[:], [P, 8], tag + "rs")
            rstd = rstd2
            pre = sb.tile([P, 512], F32, tag=tag + "pre")
            nc.vector.tensor_tensor(pre[:], cen[:],
                                    ap3(rstd, [[1, 8], [0, 64]]), op=OP.mult)
            nc.vector.tensor_tensor(pre[:], pre[:],
                                    ap3(rws, [[0, 8], [1, 64]], offset=goff),
                                    op=OP.mult)
            nc.vector.tensor_tensor(pre[:], pre[:],
                                    ap3(rws, [[0, 8], [1, 64]], offset=goff + 128),
                                    op=OP.add)
            sg = sb.tile([P, 512], F32, tag=tag + "sg")
            nc.scalar.activation(sg[:], pre[:], ACTF.Sigmoid)
            nc.vector.tensor_mul(out16[:, oslice], sg[:], pre[:])

        def batched_ln_silu(xx, gkey, bkey, out16, tag):
            go = ROWS[gkey]
            ln_silu_half(xx, 0, go, out16, slice(0, 512), tag + "a")
            ln_silu_half(xx, 512, go + 64, out16, slice(512, 1024), tag + "b")

        h1_16 = big1.tile([P, EC], F16, tag="h1_16")
        batched_ln_silu(xx1, 'g1', 'b1', h1_16, "ln1")

        # transposed layout [64, 2048]: nf cols 0-1023, ef cols 1024-2047
        h1T_ps = psT.tile([64, 2 * EC], F16, tag="hT", name=Rp + "h1T_ps")
        for t in range(TILES):
            nc.tensor.transpose(h1T_ps[:, t * P:(t + 1) * P],
                                h1_16[:, t * 64:(t + 1) * 64], ident16[:])
            nc.tensor.transpose(h1T_ps[:, EC + t * P:EC + (t + 1) * P],
                                h1_16[:, 512 + t * 64:512 + (t + 1) * 64],
                                ident16[:])
        h1T = big1.tile([64, 2 * EC], F16, tag="h1T")
        nc.scalar.copy(h1T[:], h1T_ps[:])

        xx2 = psR.tile([P, EC], F32, tag="xx", name=Rp + "xx2")
        for t in range(TILES):
            nc.tensor.matmul(xx2[:, t * 64:(t + 1) * 64],
                             h1T[:, t * P:(t + 1) * P], W2nf[:],
                             start=True, stop=True, skip_group_check=True)
            nc.tensor.matmul(xx2[:, 512 + t * 64:512 + (t + 1) * 64],
                             h1T[:, EC + t * P:EC + (t + 1) * P], W2ef[:],
                             start=True, stop=True, skip_group_check=True)

        h2_16 = big1.tile([P, EC], F16, tag="h1_16", name=Rp + "h2_16")
        batched_ln_silu(xx2, 'g2', 'b2', h2_16, "ln2")

        h2T_ps = psT.tile([64, 2 * EC], F16, tag="hT", name=Rp + "h2T_ps")
        for t in range(TILES):
            nc.tensor.transpose(h2T_ps[:, t * P:(t + 1) * P],
                                h2_16[:, t * 64:(t + 1) * 64], ident16[:])
            nc.tensor.transpose(h2T_ps[:, EC + t * P:EC + (t + 1) * P],
                                h2_16[:, 512 + t * 64:512 + (t + 1) * 64],
                                ident16[:])
        h2T = consts.tile([64, 2 * EC], F16)
        nc.scalar.copy(h2T[:], h2T_ps[:])

        # ---- phase A: projections -> proj [N, 320] fp16 ----
        for c in range(NCHUNK):
            hsT = setup.tile([128, P], F16, tag="hsT")
            (nc.scalar if c % 2 == 0 else nc.gpsimd).dma_start(
                hsT[:], T['hn_T'][0:128, c * P:(c + 1) * P])
            pa = psPA.tile([P, 672], F32, tag="pa", name=Rp + f"pa_{c}")
            nc.tensor.matmul(pa[:, 0:192], hsT[:], Wsd3[:],
                             start=True, stop=True, skip_group_check=True)
            for x in range(3):
                hvT = setup.tile([64, P], F16, tag="hvT")
                (nc.gpsimd if c % 2 == 0 else nc.scalar).dma_start(
                    hvT[:], T['hn_T'][128 + 64 * x:192 + 64 * x, c * P:(c + 1) * P])
                nc.tensor.matmul(pa[:, 192 + 160 * x:352 + 160 * x], hvT[:], Wv3[:],
                                 start=True, stop=True, skip_group_check=True)
            pev = setup.tile([P, 672], F16, tag="pev")
            nc.scalar.copy(pev[:], pa[:])
            rr = slice(c * P, (c + 1) * P)
            # src side: [s_src 64 | v0s | v1s | v2s] (cols 0-63, 192+160x..)
            dma(T['proj_s'][rr, 0:64], pev[:, 0:64])
            dma(T['proj_s'][rr, 64:160], ap3(pev, [[160, 3], [1, 32]], offset=192))
            # dst side (pre-duplicated by the matmul weight columns)
            dma(T['proj_d'][rr, :], ap3(pev, [[160, 4], [1, 128]], offset=64))

        # ---- mod table ----
        pm = psT.tile([G, 2 * S_TP], F32, tag="pm", name=Rp + "pm")
        nc.tensor.matmul(pm[:], tT[:], normWt[:], start=True, stop=True,
                         skip_group_check=True)
        msb = setup.tile([G, 2 * S_TP], F16, tag="msb")
        nc.vector.tensor_tensor(msb[:], pm[:], nbt[:], op=OP.add)
        dma(T['mod_d'][:], msb[:])

    # ---------------- dtp psum pools (8 banks total) ----------------------
    # psw Gp x2 = 2 | psred red x2 = 2 | psr reps = 2 | psN nn = 1 | psc pc = 1
    psw = ctx.enter_context(tc.tile_pool(name=Rp + "psw", bufs=2, space="PSUM"))
    psr = ctx.enter_context(tc.tile_pool(name=Rp + "psr", bufs=1, space="PSUM"))
    psred = ctx.enter_context(tc.tile_pool(name=Rp + "psred", bufs=2, space="PSUM"))
    psN = ctx.enter_context(tc.tile_pool(name=Rp + "psN", bufs=1, space="PSUM"))
    psc = ctx.enter_context(tc.tile_pool(name=Rp + "psc", bufs=1, space="PSUM"))
    sbb = ctx.enter_context(tc.tile_pool(name=Rp + "sbb", bufs=2))

    for ti in range(TILES):
        e0 = ti * P
        eix = sbg.tile([P, 3], I32, tag="eix")
        dma(eix[:], T['eidx'][e0:e0 + P, :])
        emt = sbg.tile([P, 2], F32, tag="emt")
        dma(emt[:], T['emeta'][e0:e0 + P, :])
        rvt = sbg.tile([P, 3], F16, tag="rvt")
        dma(rvt[:], T['rvecb'][e0:e0 + P, :])
        si = eix[:, 0:1]; di = eix[:, 1:2]; gi = eix[:, 2:3]
        sfc = emt[:, 0:1]; dcol = emt[:, 1:2]

        # --- gathers (fp16, full-row offset-0) ---
        g_src = sbg.tile([P, 160], F16, tag="gsrc")
        nc.gpsimd.indirect_dma_start(
            out=g_src[:], out_offset=None, in_=T['proj_s'][:],
            in_offset=IndirectOffsetOnAxis(ap=si, axis=0))
        g_dall = sbg.tile([P, 512], F16, tag="gdall")
        nc.gpsimd.indirect_dma_start(
            out=g_dall[:], out_offset=None, in_=T['proj_d'][:],
            in_offset=IndirectOffsetOnAxis(ap=di, axis=0))
        dupS = g_dall[:, 0:128]
        dupV = [g_dall[:, 128 * (x + 1):128 * (x + 2)] for x in range(3)]
        g_mod = sbg.tile([P, 2 * S_TP], F16, tag="gmod")
        nc.gpsimd.indirect_dma_start(
            out=g_mod[:], out_offset=None, in_=T['mod_d'][:],
            in_offset=IndirectOffsetOnAxis(ap=gi, axis=0))

        heR = {}
        for bi, nm in enumerate(['hes', 'hev0', 'hev1', 'hev2']):
            t16 = sbg.tile([P, 128], F16, tag="he_" + nm, name=Rp + f"he{nm}_{ti}")
            heR[nm] = t16
            dma(t16[:], T['heDup'][128 * bi:128 * (bi + 1), e0:e0 + P])

        # --- transposes into reps_ps [128, 1296] f16 (2 banks) ---
        # cols: repS 0-127 | repV0/1/2 128-511 | s1T 512-639 (r0-63 AND r64-127)
        #   | v1T-x 640+128x (each with copies at rows 0-31 / 32-63 / 64-95)
        #   | modT-shift 1024-1151 (r0-95) | modT-scale 1152-1279 (r0-95)
        #   | swc col 1280
        reps_ps = psr.tile([P, 1296], F16, tag="reps", name=Rp + f"reps_{ti}")
        nc.tensor.transpose(reps_ps[:, 0:128], dupS, ident16[:])
        for x in range(3):
            nc.tensor.transpose(reps_ps[:, 128 * (x + 1):128 * (x + 2)],
                                dupV[x], ident16[:])
        nc.tensor.transpose(reps_ps[0:64, 512:640], g_src[:, 0:64], ident16[:])
        nc.tensor.transpose(reps_ps[64:128, 512:640], g_src[:, 0:64], ident16[:])
        for x in range(3):
            for b in range(3):
                nc.tensor.transpose(
                    reps_ps[32 * b:32 * b + 32, 640 + 128 * x:768 + 128 * x],
                    g_src[:, 64 + 32 * x:96 + 32 * x], ident16[:])
        nc.tensor.transpose(reps_ps[0:96, 1024:1152], g_mod[:, 0:96], ident16[:])
        nc.tensor.transpose(reps_ps[0:96, 1152:1280], g_mod[:, 96:192], ident16[:])
        repsb = sbg.tile([P, 1024], F16, tag="repsb")
        nc.scalar.copy(repsb[:, 0:512], reps_ps[:, 0:512])
        nc.scalar.copy(repsb[:, 512:640], reps_ps[:, 512:640])
        nc.scalar.copy(repsb[0:96, 640:1024], reps_ps[0:96, 640:1024])
        modT_sh = reps_ps[0:96, 1024:1152]
        modT_sc = reps_ps[0:96, 1152:1280]
        s1b_lo = sbg.tile([64, P], F16, tag="s1b_lo")
        nc.vector.tensor_scalar(s1b_lo[:], repsb[0:64, 512:640],
                                bias_col('src_bs', 64), None, op0=OP.add)
        s1b_hi = sbg.tile([P, P], F16, tag="s1b_hi")
        nc.vector.tensor_scalar(s1b_hi[64:128, :], repsb[64:128, 512:640],
                                bcols[64:128, BCOL['src_bs']:BCOL['src_bs'] + 1],
                                None, op0=OP.add)

        def v1T(base, x):
            # copy of v1 component x at rows 32*base..32*base+32
            return repsb[32 * base:32 * base + 32, 640 + 128 * x:768 + 128 * x]

        # --- region machinery ---
        def do_region(rname, w3o, nchunks, selk, selw, nx, W3, h2row, repf,
                      red_rows, const_mm=None):
            rtile = psred.tile([P, 384], F32, tag="red",
                               name=Rp + f"red_{ti}_{rname}")
            r0, r1 = red_rows
            npairs = nchunks // 8
            for gp in range(npairs):
                G16 = sbq.tile([P, 1024], F16, tag="G16",
                               name=Rp + f"G16_{ti}_{rname}_{gp}")
                for h in range(2):
                    g = gp * 2 + h
                    c0 = g * 4
                    Gp = psw.tile([P, 512], F32, tag="Gp",
                                  name=Rp + f"Gp_{ti}_{rname}_{g}")
                    for c4 in range(4):
                        col = w3o + (c0 + c4) * P
                        nc.tensor.matmul(Gp[:, c4 * P:(c4 + 1) * P],
                                         W3[:, col:col + P], h2row,
                                         start=True, stop=True,
                                         skip_group_check=True)
                    if g % 5 == 2:
                        nc.vector.tensor_copy(G16[:, h * 512:(h + 1) * 512], Gp[:])
                    else:
                        nc.scalar.copy(G16[:, h * 512:(h + 1) * 512], Gp[:])
                for x in range(nx):
                    q = sbq.tile([P, 1024], F16, tag="q",
                                 name=Rp + f"q_{ti}_{rname}_{gp}_{x}")
                    qeng = nc.gpsimd if (x == 1 or rname == 'vs') else nc.vector
                    qeng.tensor_tensor(
                        q[:], G16[:],
                        ap3(repf(rname, x), [[0, 8], [1, P]]), op=OP.mult)
                    for c8 in range(8):
                        c = gp * 8 + c8
                        so = SEL_OFF[selk]
                        sel_ap = SEL[:, so + c * selw:so + (c + 1) * selw]
                        first = (gp == 0 and c8 == 0 and x == 0)
                        last = (gp == npairs - 1 and c8 == 7 and x == nx - 1
                                and const_mm is None)
                        nc.tensor.matmul(rtile[r0:r1, 128 * x:128 * (x + 1)],
                                         sel_ap, q[:, c8 * P:(c8 + 1) * P],
                                         start=first, stop=last,
                                         skip_group_check=True)
            if const_mm is not None:
                nc.tensor.matmul(rtile[r0:r1, 0:128], const_mm[:], h2row,
                                 start=False, stop=True, skip_group_check=True)
            return rtile

        def rep_nf(rname, x):
            if rname in ('ss', 'vs'):
                return repsb[:, 0:128]
            return repsb[:, 128 * (x + 1):128 * (x + 2)]

        def rep_ef(rname, x):
            if rname == 'ss2':
                return heR['hes'][:, :]
            return heR[f'hev{x}'][:, :]

        h2nf = h2T[:, e0:e0 + P]
        h2ef = h2T[:, EC + e0:EC + e0 + P]

        # --- dtp1 regions -> aligned bil buffers (Pool cast-copies) ---
        # bil1 [128, 1536] f16: ss c0-127 (r0-63) | sv c128-511 (r64-127)
        #   | vs c512-639 (r32-63) | v0 c640-1023 (r64-95) | v1 c1024-1407 (r0-31)
        bil1 = sbb.tile([P, 1536], F16, tag="bil1")
        bil2 = sbb.tile([S_TP, 512], F16, tag="bil2")

        r = do_region('ss', 0, 32, 'ss', 64, 1, W3nf, h2nf, rep_nf,
                      (0, 64), const_mm=C1ss)
        nc.scalar.copy(bil1[0:64, 0:128], r[0:64, 0:128])
        r = do_region('sv', 4096, 16, 'sv', 64, 3, W3nf, h2nf, rep_nf, (64, 128))
        nc.scalar.copy(bil1[64:128, 128:512], r[64:128, 0:384])
        r = do_region('vs', 6144, 16, 'vs', 32, 1, W3nf, h2nf, rep_nf,
                      (32, 64), const_mm=C1vs)
        nc.scalar.copy(bil1[32:64, 512:640], r[32:64, 0:128])
        r = do_region('v0', 8192, 8, 'v0', 32, 3, W3nf, h2nf, rep_nf, (64, 96))
        nc.scalar.copy(bil1[64:96, 640:1024], r[64:96, 0:384])
        r = do_region('v1', 9216, 8, 'v0', 32, 3, W3nf, h2nf, rep_nf, (0, 32))
        nc.scalar.copy(bil1[0:32, 1024:1408], r[0:32, 0:384])

        # --- dtp1 epilogue: fsT [96, 128] = [ss | v0], fvT [128, 128] x3
        #     with fv rows = [v1 0-31 | vs 32-63 | sv 64-127] ---
        fsT = sbq.tile([S_TP, P], F16, tag="fsT")
        nc.vector.tensor_tensor(fsT[0:64, :], s1b_lo[:], bil1[0:64, 0:128],
                                op=OP.mult)
        t32a = sbq.tile([P, P], F32, tag="t32a")
        nc.vector.tensor_tensor(t32a[64:96, :], v1T(2, 0), bil1[64:96, 640:768],
                                op=OP.mult)
        t32b = sbq.tile([P, P], F32, tag="t32b")
        nc.vector.tensor_tensor(t32b[64:96, :], v1T(2, 1), bil1[64:96, 768:896],
                                op=OP.mult)
        nc.vector.tensor_tensor(t32a[64:96, :], t32a[64:96, :], t32b[64:96, :],
                                op=OP.add)
        nc.vector.tensor_tensor(t32b[64:96, :], v1T(2, 2), bil1[64:96, 896:1024],
                                op=OP.mult)
        nc.vector.tensor_tensor(fsT[64:96, :], t32a[64:96, :], t32b[64:96, :],
                                op=OP.add)
        nc.vector.tensor_scalar(fsT[:], fsT[:], bias_col('nf_bias', 96), None,
                                op0=OP.add)
        fvT = []
        for x in range(3):
            fv = sbq.tile([P, P], F16, tag=f"fvT{x}", name=Rp + f"fvT{x}_{ti}")
            # v1-cross rows 0-31: v1T_y(0) * c_z - v1T_z(0) * c_y
            y, z = (x + 1) % 3, (x + 2) % 3
            cya = sbq.tile([32, P], F32, tag="cya")
            nc.gpsimd.tensor_tensor(cya[:], v1T(0, y),
                                    bil1[0:32, 1024 + 128 * z:1152 + 128 * z],
                                    op=OP.mult)
            cyb = sbq.tile([32, P], F32, tag="cyb")
            nc.gpsimd.tensor_tensor(cyb[:], v1T(0, z),
                                    bil1[0:32, 1024 + 128 * y:1152 + 128 * y],
                                    op=OP.mult)
            nc.vector.tensor_sub(fv[0:32, :], cya[:], cyb[:])
            # vs rows 32-63
            nc.vector.tensor_tensor(fv[32:64, :], v1T(1, x), bil1[32:64, 512:640],
                                    op=OP.mult)
            # sv rows 64-127
            nc.vector.tensor_tensor(fv[64:128, :], s1b_hi[64:128, :],
                                    bil1[64:128, 128 + 128 * x:256 + 128 * x],
                                    op=OP.mult)
            fvT.append(fv)

        # --- nt linear (nn [96, 512] f32: ns r0-63 c0-127, nv_x r64-95) ---
        nn = psN.tile([S_TP, 512], F32, tag="nn", name=Rp + f"nn_{ti}")
        nc.tensor.matmul(nn[0:64, 0:128], ntWs[:], fsT[:],
                         start=True, stop=True, skip_group_check=True)
        for x in range(3):
            nc.tensor.matmul(nn[64:96, 128 * (x + 1):128 * (x + 2)],
                             ntWv[:], fvT[x][:],
                             start=True, stop=True, skip_group_check=True)
        nsb = sbq.tile([64, P], F16, tag="nsb")
        nc.vector.tensor_scalar(nsb[:], nn[0:64, 0:128],
                                bias_col('nt_bs', 64), None, op0=OP.add)

        # --- dtp2: ss2 r0-63, v02 r64-95 ---
        r = do_region('ss2', 0, 32, 'ss', 64, 1, W3ef, h2ef, rep_ef,
                      (0, 64), const_mm=C2ss)
        nc.scalar.copy(bil2[0:64, 0:128], r[0:64, 0:128])
        r = do_region('v02', 4096, 8, 'v0', 32, 3, W3ef, h2ef, rep_ef, (64, 96))
        nc.scalar.copy(bil2[64:96, 128:512], r[64:96, 0:384])

        # --- dtp2 epilogue -> asT [96, 128] fp16 ---
        asT = sbq.tile([S_TP, P], F16, tag="asT")
        nc.vector.tensor_tensor(asT[0:64, :], nsb[:], bil2[0:64, 0:128],
                                op=OP.mult)
        u32a = sbq.tile([S_TP, P], F32, tag="u32a")
        nc.vector.tensor_tensor(u32a[64:96, :], nn[64:96, 128:256],
                                bil2[64:96, 128:256], op=OP.mult)
        u32b = sbq.tile([S_TP, P], F32, tag="u32b")
        nc.vector.tensor_tensor(u32b[64:96, :], nn[64:96, 256:384],
                                bil2[64:96, 256:384], op=OP.mult)
        nc.vector.tensor_tensor(u32a[64:96, :], u32a[64:96, :], u32b[64:96, :],
                                op=OP.add)
        nc.vector.tensor_tensor(u32b[64:96, :], nn[64:96, 384:512],
                                bil2[64:96, 384:512], op=OP.mult)
        nc.vector.tensor_tensor(asT[64:96, :], u32a[64:96, :], u32b[64:96, :],
                                op=OP.add)
        nc.vector.tensor_scalar(asT[:], asT[:], bias_col('ef_bias', 96), None,
                                op0=OP.add)

        # --- adaLN (transposed, stats via ones-matmuls into statp) ---
        sqT = sbq.tile([S_TP, P], F16, tag="sqT")
        nc.scalar.activation(sqT[:], asT[:], ACTF.Square)
        statp = psw.tile([P, 512], F32, tag="Gp", name=Rp + f"statp_{ti}")
        nc.tensor.matmul(statp[0:1, 0:128], ones96[:], asT[:],
                         start=True, stop=True, skip_group_check=True)
        nc.tensor.matmul(statp[0:1, 128:256], ones96[:], sqT[:],
                         start=True, stop=True, skip_group_check=True)
        mu_r = sbq.tile([1, P], F32, tag="mu_r")
        nc.vector.tensor_scalar_mul(mu_r[:], statp[0:1, 0:128], 1.0 / S_TP)
        musq = sbq.tile([1, P], F32, tag="musq")
        nc.vector.tensor_mul(musq[:], mu_r[:], mu_r[:])
        var_r = sbq.tile([1, P], F32, tag="var_r")
        nc.vector.tensor_scalar(var_r[:], statp[0:1, 128:256], 1.0 / S_TP, None,
                                op0=OP.mult)
        nc.vector.tensor_sub(var_r[:], var_r[:], musq[:])
        rstd_r = sbq.tile([1, P], F32, tag="rstd_r")
        rsqrt_ops(nc.vector, sbq, rstd_r[:], var_r[:], [1, P], "ars")
        muB = sbq.tile([S_TP, P], F32, tag="muB")
        nc.gpsimd.partition_broadcast(muB[:], mu_r[:])
        rstdB = sbq.tile([S_TP, P], F32, tag="rstdB")
        nc.gpsimd.partition_broadcast(rstdB[:], rstd_r[:])
        snT = sbq.tile([S_TP, P], F16, tag="snT")
        nc.vector.tensor_tensor(snT[:], asT[:], muB[:], op=OP.subtract)
        nc.vector.tensor_mul(snT[:], snT[:], rstdB[:])
        nc.vector.tensor_tensor(snT[:], snT[:], modT_sc, op=OP.mult)
        nc.vector.tensor_tensor(snT[:], snT[:], modT_sh, op=OP.add)

        # --- scalar head (hd in pc c48-175 r0-31; sw in statp c256-383) ---
        pc = psc.tile([P, 224], F32, tag="pc", name=Rp + f"pc_{ti}")
        nc.tensor.matmul(pc[0:32, 48:176], spW1[:], snT[:],
                         start=True, stop=True, skip_group_check=True)
        hd16 = sbq.tile([32, P], F16, tag="hd16")
        nc.vector.tensor_scalar(hd16[:], pc[0:32, 48:176],
                                bias_col('sp_b1', 32), None, op0=OP.add)
        sg16 = sbq.tile([32, P], F16, tag="sg16")
        nc.scalar.activation(sg16[:], hd16[:], ACTF.Sigmoid)
        nc.vector.tensor_mul(hd16[:], hd16[:], sg16[:])
        nc.tensor.matmul(statp[0:1, 256:384], spW2c[:], hd16[:],
                         start=True, stop=True, skip_group_check=True)
        sw_sb = sbq.tile([1, P], F16, tag="sw_sb")
        nc.vector.tensor_scalar(sw_sb[:], statp[0:1, 256:384], sp_b2[0:1, 0:1],
                                None, op0=OP.add)
        nc.tensor.transpose(reps_ps[:, 1280:1281], sw_sb[:], ident16[0:1, 0:1])

        # --- force + scatter (acc region: pc c0-47) ---
        den = sbq.tile([P, 1], F32, tag="den")
        nc.vector.scalar_tensor_tensor(den[:], dcol, 1.0, dcol,
                                       op0=OP.add, op1=OP.mult)
        nc.vector.reciprocal(den[:], den[:])
        coef = sbq.tile([P, 1], F32, tag="coef")
        nc.vector.tensor_tensor(coef[:], reps_ps[:, 1280:1281], den[:],
                                op=OP.mult)
        frc = sbq.tile([P, 3], F16, tag="frc")
        nc.vector.tensor_scalar(frc[:], rvt[:], coef[:, :1], None, op0=OP.mult)
        oh = sboh.tile([P, N], F16, tag="oh")
        nc.vector.tensor_scalar(oh[:], iota16[:], sfc, None, op0=OP.is_equal)
        for ch in range(NCHUNK):
            nc.tensor.matmul(pc[:, 3 * ch:3 * ch + 3],
                             oh[:, ch * P:(ch + 1) * P], frc[:],
                             start=True, stop=True, skip_group_check=True)
        nc.vector.tensor_add(acc_sb[:], acc_sb[:], pc[:, 0:48])

    # ---------------- final: write accumulator ----------------
    for ch in range(NCHUNK):
        dma(T['out'][ch * P:(ch + 1) * P, :], acc_sb[:, 3 * ch:3 * ch + 3])


# ======================= host side =======================

def host_prep(inp):
    inp = {k: np.asarray(v) for k, v in inp.items()}
    src = inp['edge_index'][0].astype(np.int32)
    dst = inp['edge_index'][1].astype(np.int32)
    perm = np.argsort(src, kind='stable')
    src, dst = src[perm], dst[perm]
    gid = inp['batch'].astype(np.int32)[src]
    h_edge = inp['h_edge'][perm].astype(np.float32)
    dist = inp['distance'][perm].astype(np.float32)
    rvec = inp['relative_vec'][perm].astype(np.float32)

    f16 = np.float16

    # -- radial weights --
    std = inp['rbf_std'].astype(np.float64)
    mean = inp['rbf_mean'].astype(np.float64)
    acol = (float(inp['rbf_w']) / CUTOFF) / std
    bcol = (float(inp['rbf_b']) - mean) / std
    cc = 1.0 / (np.sqrt(2 * np.pi) * std)
    W1p = np.concatenate([inp['nf_W1'], inp['ef_W1']], axis=1) * cc[:, None]

    # -- W3 folds --
    s8 = 1.0 / np.sqrt(64.0)        # = 1/8 (ss, vs)
    ssv = 1.0 / np.sqrt(32.0)
    sv0 = 1.0 / np.sqrt(96.0)
    sv1 = 1.0 / np.sqrt(64.0)
    W3 = inp['nf_W3'].astype(np.float64)
    W3nf = np.concatenate([
        W3[:, 0:4096] * s8, W3[:, 4096:6144] * ssv, W3[:, 6144:8192] * s8,
        W3[:, 8192:9216] * sv0, W3[:, 9216:10240] * sv1], axis=1)
    dst_bs = inp['dst_bs'].astype(np.float64)
    C1ss = np.einsum('kuv,v->ku', (W3[:, 0:4096] * s8).reshape(64, 64, 64), dst_bs)
    C1vs = np.einsum('kuv,v->ku', (W3[:, 6144:8192] * s8).reshape(64, 32, 64), dst_bs)

    W3e = inp['ef_W3'].astype(np.float64)
    etWs_s = inp['et_Ws'].astype(np.float64) / 8.0       # [c, v]
    etWv_s = inp['et_Wv'].astype(np.float64) / np.sqrt(32.0)
    et_bs = inp['et_bs'].astype(np.float64)
    W3e_ss = (W3e[:, 0:4096] * s8).reshape(64, 64, 64)
    W3e_v0 = (W3e[:, 8192:9216] * sv0).reshape(64, 32, 32)
    W3ef_ss2 = np.einsum('kuv,cv->kuc', W3e_ss, etWs_s).reshape(64, 4096)
    W3ef_v02 = np.einsum('kuv,cv->kuc', W3e_v0, etWv_s).reshape(64, 1024)
    W3ef = np.concatenate([W3ef_ss2, W3ef_v02], axis=1)
    C2ss = np.einsum('kuv,v->ku', W3e_ss, et_bs)

    # -- selection matrices --
    SEL = np.zeros((128, SEL_W), np.float32)
    for c in range(32):   # ss: 2u x 64v
        for p in range(128):
            SEL[p, SEL_OFF['ss'] + c * 64 + 2 * c + p // 64] = 1
    for c in range(16):   # sv: 4u x 32v
        for p in range(128):
            SEL[p, SEL_OFF['sv'] + c * 64 + 4 * c + p // 32] = 1
    for c in range(16):   # vs: 2u x 64v
        for p in range(128):
            SEL[p, SEL_OFF['vs'] + c * 32 + 2 * c + p // 64] = 1
    for c in range(8):    # v0/v1: 4u x 32v
        for p in range(128):
            SEL[p, SEL_OFF['v0'] + c * 32 + 4 * c + p // 32] = 1

    # -- misc consts --
    rows = np.zeros(RWID, np.float32)
    rows[0:64] = inp['nf_g1']; rows[64:128] = inp['ef_g1']
    rows[128:192] = inp['nf_b1']; rows[192:256] = inp['ef_b1']
    rows[256:320] = inp['nf_g2']; rows[320:384] = inp['ef_g2']
    rows[384:448] = inp['nf_b2']; rows[448:512] = inp['ef_b2']
    bcols = np.zeros((128, 8), np.float32)
    bcols[0:96, BCOL['nf_bias']] = inp['nf_bias']
    bcols[0:96, BCOL['ef_bias']] = inp['ef_bias']
    bcols[0:64, BCOL['nt_bs']] = inp['nt_bs']
    bcols[0:64, BCOL['src_bs']] = inp['src_bs']
    bcols[64:128, BCOL['src_bs']] = inp['src_bs']
    bcols[0:32, BCOL['sp_b1']] = inp['sp_b1']
    normbt = inp['norm_bt'][:2 * S_TP].astype(np.float32).copy()
    normbt[S_TP:] += 1.0

    # -- node features (transposed, v deinterleaved) --
    hn = inp['h_node'].astype(np.float32)
    hv = hn[:, 128:].reshape(N, 64, 3)
    hn_T = np.concatenate([hn[:, 0:128].T, hv[:, :, 0].T, hv[:, :, 1].T,
                           hv[:, :, 2].T], axis=0)
    hev = h_edge[:, 64:].reshape(E, 32, 3)
    hes_T = h_edge[:, 0:64].T
    heDup_full = np.concatenate(
        [hes_T, hes_T] + [hev[:, :, x].T for x in range(3) for _ in range(4)],
        axis=0)   # [512, E]: hes x2 | hev0 x4 | hev1 x4 | hev2 x4 (x outer)

    shared = dict(
        hn_T=hn_T.astype(f16),
        t_T=np.ascontiguousarray(inp['t'].T).astype(np.float32),
        normWt=np.ascontiguousarray(inp['norm_Wt'][:, :2 * S_TP]).astype(np.float32),
        normbt=normbt.reshape(1, -1),
        rows=rows.reshape(1, -1),
        biascols=bcols,
        acol=acol.reshape(-1, 1).astype(np.float32),
        bcol=bcol.reshape(-1, 1).astype(np.float32),
        W1p=W1p.astype(f16),
        W2nf=inp['nf_W2'].astype(f16), W2ef=inp['ef_W2'].astype(f16),
        W3nf=W3nf.astype(f16), W3ef=W3ef.astype(f16),
        C1ss=C1ss.astype(f16), C1vs=C1vs.astype(f16), C2ss=C2ss.astype(f16),
        SEL=SEL.astype(f16),
        Wsd3=(np.concatenate([inp['src_Ws'], inp['dst_Ws'], inp['dst_Ws']],
                             axis=1) / np.sqrt(128.0)).astype(f16),
        Wv3=(np.concatenate([inp['src_Wv']] + [inp['dst_Wv']] * 4,
                            axis=1) / np.sqrt(64.0)).astype(f16),
        ntWs=(inp['nt_Ws'] / np.sqrt(96.0)).astype(f16),
        ntWv=(np.concatenate([inp['nt_Wv'][96:128], inp['nt_Wv'][64:96],
                              inp['nt_Wv'][0:64]], axis=0)
              / np.sqrt(128.0)).astype(f16),
        spW1=(inp['sp_W1'] / np.sqrt(96.0)).astype(f16),
        spW2c=(inp['sp_W2'] / np.sqrt(32.0)).astype(f16),
        sp_b2=inp['sp_b2'].reshape(1, 1).astype(np.float32),
        iotaR=np.arange(N, dtype=f16).reshape(1, -1),
    )
    shared = {k: np.ascontiguousarray(v) for k, v in shared.items()}

    in_maps = []
    for c in range(NC_CORES):
        sl = slice(c * EC, (c + 1) * EC)
        m = dict(shared)
        m['heDup'] = np.ascontiguousarray(heDup_full[:, sl]).astype(f16)
        m['distR'] = dist[sl].reshape(1, -1)
        m['rvecb'] = rvec[sl].astype(f16)
        m['eidx'] = np.ascontiguousarray(
            np.stack([src[sl], dst[sl], gid[sl]], axis=1).astype(np.int32))
        m['emeta'] = np.ascontiguousarray(
            np.stack([src[sl].astype(np.float32), dist[sl]], axis=1))
        in_maps.append(m)
    return in_maps


_CACHED_NC = None


def kernel(**inputs):
    global _CACHED_NC
    from concourse.bass_utils import run_bass_kernel_spmd
    if _CACHED_NC is None:
        _CACHED_NC = build_nc()
    in_maps = host_prep(inputs)
    res = run_bass_kernel_spmd(_CACHED_NC, in_maps, list(range(NC_CORES)))
    out = np.zeros((N, 3), np.float32)
    for r in res.results:
        out += r['out']
    return out
